# revision 46
# baseline (speedup 1.0000x reference)
"""DNC memory-controller step (nn_Controller_85332410237553) on 8 trn2 cores.

Data-parallel: core k handles batches 4k..4k+3. Heavy op is streaming
link [b,1,1024,1024] through two small matmuls (rw @ link_new and
rw @ link_new^T) which decompose algebraically onto raw `link`:
  bwd[r,j] = (rwL - wrwL)[r,j] - w_j*rwL[r,j] + (rw.w)*p_j - rw[r,j]*w_j*p_j
  fwd[r,i] = (1-w_i)*Lrw[r,i] - Lwrw[r,i] + w_i*(rw.p) - rw[r,i]*w_i*p_i
Both products are computed with link tiles as the stationary matmul
operand and the 8 rw columns moving, so outputs land directly in
c-partitioned layout. fwd needs link^T tiles: PE-transpose 128x128
blocks, pack 4 per PSUM bank, evacuate with DVE/ACT alternating.
All per-(b,r) scalar work is batched across the 4 local batches with
broadcast access patterns; softmax partition sums use ones-matmuls.
Single activation table (exp/ln/copy/square; sigmoid via exp).
"""
import types
from contextlib import ExitStack

import numpy as np

B, C, D, R, NW = 32, 1024, 64, 4, 1
NC = 8          # cores
BB = B // NC    # batches per core = 4
K = C // 128    # 8 c-chunks
EPS = 1e-6
HX, HW_IF = 512, 796

# ---- scalar-column indices in S_pre [4(b), NS] ------------------------
S_FG = 0            # 4: sigmoid free_gate r=0..3
S_GAG = 4           # 1: ga*gw
S_GAG2 = 5          # 1: (1-ga)*gw
S_WS = 6            # 1: softplus write_strength
S_RS = 7            # 4: softplus read_strengths
S_AS = 11           # 1: softplus alloc_strength
S_NAS = 12          # 1: -softplus alloc_strength
S_MS = 13           # 4: softplus mode_strengths r
S_PIE = 17          # 12: normalized read_mode (r,m) m in (bwd,cnt,fwd)
S_KNW = 29          # 1: ||wmask*wkey||
S_KNR = 30          # 4: ||rmask_r*rkey_r||
NS = 34

ACT_TABLE = "natural_log_exp_and_others"


def build_program():
    import concourse.bass as bass
    import concourse.bacc as bacc
    import concourse.mybir as mybir
    import concourse.tile as tile

    dt = mybir.dt
    f32 = dt.float32

    nc = bacc.Bacc("TRN2", target_bir_lowering=False, debug=False,
                   num_devices=NC)

    # keep exp+ln+copy+square in one resident activation table: restrict
    # the placement pass to the combined set (ids stay act_info indices)
    def _atl(self):
        from concourse.hw_specs import get_activation_tables
        import bass_rust as _br
        has_act = any(isinstance(i, mybir.InstActivation)
                      for b in self.main_func.blocks for i in b.instructions)
        if not has_act:
            return
        tables = [(n, (s if n == ACT_TABLE else set()))
                  for n, s in get_activation_tables(self.m.arch).items()]
        _br.insert_act_table_loads(self, tables)
    nc.insert_act_table_loads = types.MethodType(_atl, nc)

    din = {}
    def dram_in(name, shape, dtype=f32):
        din[name] = nc.dram_tensor(name, list(shape), dtype,
                                   kind="ExternalInput").ap()
    dram_in("x", (BB, HX))
    dram_in("W_if", (HX, HW_IF), dt.float32r)
    dram_in("b_if", (HW_IF,), dt.float32r)
    dram_in("memory", (BB, C, D))
    dram_in("usage", (BB, C))
    dram_in("link", (BB, C, C), dt.float32r)
    dram_in("precedence", (BB, C))
    dram_in("prw", (BB, R, C))
    dram_in("pww", (BB, C))
    dram_in("cpack", (128, 768), dt.float32r)  # eye | ones | selb(rows 0:4)
    out_rv = nc.dram_tensor("read_vectors", [BB, R, D], f32,
                            kind="ExternalOutput").ap()

    with tile.TileContext(nc) as tc:
        with ExitStack() as ctx:
            emit(ctx, tc, nc, din, out_rv, bass, mybir, tile)
    nc.compile()
    return nc


def emit(ctx, tc, nc, din, out_rv, bass, mybir, tile):
    dt = mybir.dt
    f32, f32r = dt.float32, dt.float32r
    AF = mybir.ActivationFunctionType
    ALU = mybir.AluOpType
    AX = mybir.AxisListType
    AP = bass.AP
    V, S, T, SY, P = nc.vector, nc.scalar, nc.tensor, nc.sync, nc.gpsimd

    def _ap(base):
        return base if isinstance(base, AP) else base[:]

    def A(base, off, dims):
        # keep base partition dim, replace free dims
        b = _ap(base)
        return AP(b.tensor, b.offset + off, [list(b.ap[0])] + dims)

    def M(base, off, dims):
        # fully manual AP (dims[0] is the partition dim)
        b = _ap(base)
        return AP(b.tensor, b.offset + off, dims)

    pc = ctx.enter_context(tc.tile_pool(name="const", bufs=1))
    pp = ctx.enter_context(tc.tile_pool(name="prep", bufs=1))
    plink = ctx.enter_context(tc.tile_pool(name="link", bufs=2))
    plt = ctx.enter_context(tc.tile_pool(name="lt", bufs=2))
    pmem = ctx.enter_context(tc.tile_pool(name="mem", bufs=2))
    pw = ctx.enter_context(tc.tile_pool(name="work", bufs=1))
    psc = ctx.enter_context(tc.tile_pool(name="scratch", bufs=4))
    ptp = ctx.enter_context(tc.tile_pool(name="tpsum", bufs=1, space="PSUM"))
    plp = ctx.enter_context(tc.tile_pool(name="lpsum", bufs=1, space="PSUM"))
    psmall = ctx.enter_context(tc.tile_pool(name="spsum", bufs=1, space="PSUM"))

    # ---------------- constants / weights / inputs ----------------
    cpack_t = pc.tile([128, 768], f32r); SY.dma_start(cpack_t[:], din["cpack"])
    eyer = cpack_t[:, 0:128]
    eye = eyer.bitcast(f32)
    ones = cpack_t[:, 128:256].bitcast(f32)
    ones1r = cpack_t[0:1, 128:128+BB]            # f32r ones row [1, BB]
    def selb_l(b):
        return cpack_t[0:BB, 256+b*128:256+(b+1)*128].bitcast(f32)
    x_nat = pp.tile([BB, HX], f32); SY.dma_start(x_nat[:], din["x"])
    W_sb = pc.tile([128, 4 * HW_IF], f32r)
    SY.dma_start(W_sb[:], M(din["W_if"], 0,
                            [[HW_IF, 128], [128*HW_IF, 4], [1, HW_IF]]))
    b_sb = pc.tile([1, HW_IF], f32r)
    SY.dma_start(b_sb[:], M(din["b_if"], 0, [[HW_IF, 1], [1, HW_IF]]))
    # small cP inputs
    nats = {}
    for name in ("usage", "pww", "precedence"):
        nat = pp.tile([BB, C], f32, tag=f"nat_{name}", name=f"nat_{name}")
        SY.dma_start(nat[:], din[name])
        nats[name] = nat
    rw_nat = pp.tile([BB*R, C], f32)
    SY.dma_start(rw_nat[:], M(din["prw"], 0, [[C, BB*R], [1, C]]))
    # memory loads (needed early for pre-link of b0)
    Mball = pc.tile([128, BB * K * D], f32)
    SY.dma_start(Mball[:], M(din["memory"], 0,
                             [[D, 128], [C*D, BB], [128*D, K], [1, D]]))
    Mbs = [Mball[:, b*K*D:(b+1)*K*D] for b in range(BB)]
    # link streams for b0/b1 queued right after
    def load_ln(b):
        LNb = []
        for ih in range(K):
            t = plink.tile([128, C], f32r, tag=f"lnat{ih}", bufs=2,
                           name=f"ln{b}_{ih}")
            SY.dma_start(t[:], M(din["link"], b*C*C + ih*128*C,
                                 [[C, 128], [1, C]]))
            LNb.append(t)
        return LNb
    LN_all = [load_ln(0), load_ln(1), None, None]

    def pet(in_ap, np_, nf_):
        # PE transpose [np_, nf_] -> psum [nf_, np_] (f32)
        ps = ptp.tile([nf_, np_], f32, tag="tp", bufs=2)
        T.transpose(ps[:], in_ap, eye[:np_, :np_])
        return ps

    # ---------------- interface projection y = x@W + b ----------------
    xT = pp.tile([128, 4 * BB], f32r)
    for k in range(4):
        ps = pet(x_nat[:, k*128:(k+1)*128], BB, 128)
        S.copy(xT[:, k*BB:(k+1)*BB], ps[:])
    y = pp.tile([BB, HW_IF], f32)
    y_ps0 = psmall.tile([BB, 512], f32, tag="sp")
    for k in range(4):
        T.matmul(y_ps0[:], xT[:, k*BB:(k+1)*BB],
                 W_sb[:, k*HW_IF:k*HW_IF+512],
                 start=(k == 0), stop=False)
    T.matmul(y_ps0[:], ones1r, b_sb[:, 0:512], start=False, stop=True)
    V.tensor_copy(y[:, 0:512], y_ps0[:])
    y_ps1 = psmall.tile([BB, HW_IF - 512], f32, tag="sp")
    for k in range(4):
        T.matmul(y_ps1[:], xT[:, k*BB:(k+1)*BB],
                 W_sb[:, k*HW_IF+512:(k+1)*HW_IF],
                 start=(k == 0), stop=False)
    T.matmul(y_ps1[:], ones1r, b_sb[:, 512:HW_IF], start=False, stop=True)
    V.tensor_copy(y[:, 512:HW_IF], y_ps1[:])

    # ---------------- heads (sigmoid via exp) ----------------
    sig1 = pp.tile([BB, 70], f32)   # ev(64) fg(4) ga gw  <- y[64:134]
    sig2 = pp.tile([BB, 320], f32)  # rmask(256) wmask(64) <- y[472:792]
    S.activation(sig1[:], y[:, 64:134], AF.Exp, scale=-1.0)
    S.activation(sig2[:], y[:, 472:792], AF.Exp, scale=-1.0)
    V.tensor_scalar(sig1[:], sig1[:], 1.0, None, ALU.add)
    V.tensor_scalar(sig2[:], sig2[:], 1.0, None, ALU.add)
    V.reciprocal(sig1[:], sig1[:])
    V.reciprocal(sig2[:], sig2[:])
    ev = sig1[:, 0:64]
    rmask, wmask = sig2[:, 0:256], sig2[:, 256:320]
    wv = y[:, 0:64]
    wkey, rkey = y[:, 146:210], y[:, 211:467]

    S_pre = pp.tile([BB, NS], f32)
    S.copy(S_pre[:, S_FG:S_FG+4], sig1[:, 64:68])
    V.tensor_mul(S_pre[:, S_GAG:S_GAG+1], sig1[:, 68:69], sig1[:, 69:70])
    V.tensor_sub(S_pre[:, S_GAG2:S_GAG2+1], sig1[:, 69:70], S_pre[:, S_GAG:S_GAG+1])
    spx = pp.tile([BB, 10], f32)   # softplus = ln(1 + exp(x))
    S.activation(spx[:, 0:1], y[:, 210:211], AF.Exp)
    S.activation(spx[:, 1:6], y[:, 467:472], AF.Exp)
    S.activation(spx[:, 6:10], y[:, 792:796], AF.Exp)
    S.activation(S_pre[:, S_WS:S_WS+1], spx[:, 0:1], AF.Ln, bias=1.0)
    S.activation(S_pre[:, S_RS:S_RS+5], spx[:, 1:6], AF.Ln, bias=1.0)
    S.activation(S_pre[:, S_MS:S_MS+4], spx[:, 6:10], AF.Ln, bias=1.0)
    V.tensor_scalar_mul(S_pre[:, S_NAS:S_NAS+1], S_pre[:, S_AS:S_AS+1], -1.0)
    # read mode softmax
    pie = pp.tile([BB, 12], f32)
    S.activation(pie[:], y[:, 134:146], AF.Exp)
    piZ = pp.tile([BB, 4], f32)
    V.tensor_reduce(piZ[:], A(pie, 0, [[3, 4], [1, 3]]), AX.X, ALU.add)
    piZr = pp.tile([BB, 4], f32)
    V.reciprocal(piZr[:], piZ[:])
    for m in range(3):
        V.tensor_tensor(A(S_pre, S_PIE+m, [[3, 4]]), A(pie, m, [[3, 4]]),
                        piZr[:], ALU.mult)
    # masked keys + norms
    wmk = pp.tile([BB, 64], f32);  V.tensor_mul(wmk[:], wmask, wkey)
    wmk2 = pp.tile([BB, 64], f32); V.tensor_mul(wmk2[:], wmk[:], wmask)
    wm2 = pp.tile([BB, 64], f32);  V.tensor_mul(wm2[:], wmask, wmask)
    rmk = pp.tile([BB, 256], f32); V.tensor_mul(rmk[:], rmask, rkey)
    rmk2 = pp.tile([BB, 256], f32); V.tensor_mul(rmk2[:], rmk[:], rmask)
    rm2 = pp.tile([BB, 256], f32); V.tensor_mul(rm2[:], rmask, rmask)
    sq = pp.tile([BB, 256], f32)
    V.tensor_mul(sq[:, 0:64], wmk[:], wmk[:])
    kn2 = pp.tile([BB, 5], f32)
    V.tensor_reduce(kn2[:, 0:1], sq[:, 0:64], AX.X, ALU.add)
    V.tensor_mul(sq[:], rmk[:], rmk[:])
    V.tensor_reduce(kn2[:, 1:5], A(sq, 0, [[64, 4], [1, 64]]), AX.X, ALU.add)
    S.activation(kn2[:], kn2[:], AF.Ln)
    S.activation(S_pre[:, S_KNW:S_KNW+5], kn2[:], AF.Exp, scale=0.5)  # sqrt

    # ---------------- replicated per-b scalars: SALL [128, 4*NS] ----------
    ps_sall = psmall.tile([128, BB * NS], f32, tag="sp")
    for b in range(BB):
        T.matmul(ps_sall[:, b*NS:(b+1)*NS], selb_l(b),
                 S_pre[:], start=True, stop=True)
    SALL_t = pw.tile([128, BB * NS], f32)
    V.tensor_copy(SALL_t[:], ps_sall[:])
    def scol(b, c):
        return SALL_t[:, b*NS+c:b*NS+c+1]

    # replicated ev|wv rows per b: ROWR [128, 512] col = b*128 + (ev | wv)
    ROWS_pre = pp.tile([BB, 128], f32)
    S.copy(ROWS_pre[:, 0:64], ev)
    S.copy(ROWS_pre[:, 64:128], wv)
    ps_rowr = psmall.tile([128, BB * 128], f32, tag="sp")
    for b in range(BB):
        T.matmul(ps_rowr[:, b*128:(b+1)*128], selb_l(b),
                 ROWS_pre[:], start=True, stop=True)
    ROWR = pw.tile([128, BB * 128], f32)
    V.tensor_copy(ROWR[:], ps_rowr[:])

    # proj-matmul rhs: [128, 2] per b (rows 0:64 key-col, 64:128 mask2-col)
    RHSW = pw.tile([128, 2 * BB], f32)
    V.memset(RHSW[:], 0.0)
    ps = pet(wmk2[:], BB, 64)       # [64, BB]
    S.copy(M(RHSW, 0, [[2*BB, 64], [2, BB]]), ps[:])
    ps = pet(wm2[:], BB, 64)
    S.copy(M(RHSW, 64*2*BB + 1, [[2*BB, 64], [2, BB]]), ps[:])
    RHSR = pw.tile([128, 8 * BB], f32)
    V.memset(RHSR[:], 0.0)
    for r in range(R):
        ps = pet(rmk2[:, r*64:(r+1)*64], BB, 64)
        S.copy(M(RHSR, r, [[8*BB, 64], [8, BB]]), ps[:])
        ps = pet(rm2[:, r*64:(r+1)*64], BB, 64)
        S.copy(M(RHSR, 64*8*BB + 4 + r, [[8*BB, 64], [8, BB]]), ps[:])

    MTSs = [None] * BB

    def pre_mts(b):
        Mb = Mbs[b]
        # ---- MTS stack (memT | memT^2) -----------------------------------
        MTS = pmem.tile([128, C], f32, tag="mts", name=f"mts{b}", bufs=4)
        for g in range(2):
            psg = ptp.tile([64, 512], f32, tag="tp", bufs=2, name=f"mtp{b}_{g}")
            for q in range(4):
                k = g*4 + q
                T.transpose(psg[:, q*128:(q+1)*128], Mb[:, k*D:(k+1)*D],
                            eye[:])
            (V.tensor_copy if g == 0 else S.copy)(
                MTS[0:64, g*512:(g+1)*512], psg[:])
        S.square(M(MTS, 64*C, [[C, 64], [1, C]]), MTS[0:64, :])
        MTSs[b] = MTS

    # ---------------- cP tiles [128,32] (b,k) + rwT -----------------------
    def to_cp32(name):
        nat = nats[name]
        t = pw.tile([128, 32], f32, tag=f"cp_{name}")
        for k in range(K):
            ps = pet(nat[:, k*128:(k+1)*128], BB, 128)
            S.copy(M(t, k, [[32, 128], [8, BB]]), ps[:])
        return t
    u32 = to_cp32("usage")
    pw32 = to_cp32("pww")
    p32 = to_cp32("precedence")
    rwT = pw.tile([128, 128], f32)   # col = b*32 + r*8 + k
    for k in range(K):
        ps = pet(rw_nat[:, k*128:(k+1)*128], BB*R, 128)  # [128, 16(b,r)]
        V.tensor_copy(M(rwT, k, [[128, 128], [32, BB], [8, R]]),
                      M(ps, 0, [[16, 128], [4, BB], [1, R]]))

    # ---------------- usage update (all-b) --------------------------------
    t1 = psc.tile([128, 32], f32, tag="t1")
    u1 = pw.tile([128, 32], f32)
    V.tensor_mul(t1[:], u32[:], pw32[:])
    V.tensor_add(u1[:], u32[:], pw32[:])
    V.tensor_sub(u1[:], u1[:], t1[:])
    M4 = pw.tile([128, 128], f32)
    P.tensor_tensor(M4[:], rwT[:],
                    A(SALL_t, S_FG, [[NS, BB], [1, R], [0, K]]), ALU.mult)
    P.tensor_scalar(M4[:], M4[:], 1.0, None, ALU.subtract)
    phi2 = psc.tile([128, 64], f32, tag="phi2")
    V.tensor_tensor(phi2[:], A(M4, 0, [[32, BB], [16, 2], [1, K]]),
                    A(M4, 8, [[32, BB], [16, 2], [1, K]]), ALU.mult)
    phi = psc.tile([128, 32], f32, tag="phi")
    V.tensor_tensor(phi[:], A(phi2, 0, [[16, BB], [1, K]]),
                    A(phi2, 8, [[16, BB], [1, K]]), ALU.mult)
    u2 = pw.tile([128, 32], f32)
    V.tensor_mul(u2[:], u1[:], phi[:])

    # pdots partials: PARTSD col = b*8 + (pd r | wd r)
    PARTSD = pw.tile([128, 32], f32)
    PRt = psc.tile([128, 128], f32, tag="prt", bufs=2)
    P.tensor_tensor(PRt[:], rwT[:],
                    A(p32, 0, [[8, BB], [0, R], [1, K]]), ALU.mult)
    V.tensor_reduce(A(PARTSD, 0, [[8, BB], [1, R]]),
                    A(PRt, 0, [[8, 16], [1, 8]]), AX.X, ALU.add)

    epsc = pw.tile([128, 1], f32, tag="epsc")
    V.memset(epsc[:], EPS)

    # ============== software-pipelined batch loop =========================
    Mns = [None] * BB
    W32s = [None] * BB
    X8s = [None] * BB
    DOTSs = [None] * BB
    WPAs = [None] * BB
    RCbs = [None] * BB

    def pre_batch_a(b):
        pre_mts(b)
        MTS = MTSs[b]
        # ---- write-content proj + w32 ------------------------------------
        psW = psmall.tile([128, 16], f32, tag="sp")
        for k in range(K):
            T.matmul(psW[:, k*2:k*2+2], MTS[:, k*128:(k+1)*128],
                     RHSW[:, b*2:(b+1)*2], start=True, stop=True)
        mnwb = psc.tile([128, 8], f32, tag="mnwb", bufs=2)
        S.activation(mnwb[:], A(psW, 1, [[2, K]]), AF.Ln)
        S.activation(mnwb[:], mnwb[:], AF.Exp, scale=0.5)
        V.tensor_scalar(mnwb[:], mnwb[:], scol(b, S_KNW), EPS,
                        ALU.mult, ALU.add)
        V.reciprocal(mnwb[:], mnwb[:])
        EWA = psc.tile([128, 16], f32, tag="ewa", bufs=2)  # ea 0:8 | ew 8:16
        simwb = psc.tile([128, 8], f32, tag="simwb", bufs=2)
        V.tensor_tensor(simwb[:], A(psW, 0, [[2, K]]), mnwb[:], ALU.mult)
        S.activation(EWA[:, 0:8], u2[:, b*8:(b+1)*8], AF.Exp,
                     bias=scol(b, S_AS), scale=scol(b, S_NAS))
        S.activation(EWA[:, 8:16], simwb[:], AF.Exp, scale=scol(b, S_WS))
        psZw = psmall.tile([128, 16], f32, tag="sp")
        T.matmul(psZw[:], ones, EWA[:], start=True, stop=True)
        Z2 = psc.tile([128, 2], f32, tag="z2", bufs=2)
        V.tensor_reduce(Z2[:], A(psZw, 0, [[8, 2], [1, 8]]), AX.X, ALU.add)
        V.reciprocal(Z2[:], Z2[:])
        w32b = pw.tile([128, 8], f32, tag=f"w32_{b}")
        tb2 = psc.tile([128, 8], f32, tag="tb2", bufs=2)
        V.tensor_scalar(w32b[:], EWA[:, 0:8], Z2[:, 0:1],
                        scol(b, S_GAG), ALU.mult, ALU.mult)
        V.tensor_scalar(tb2[:], EWA[:, 8:16], Z2[:, 1:2],
                        scol(b, S_GAG2), ALU.mult, ALU.mult)
        V.tensor_add(w32b[:], w32b[:], tb2[:])
        W32s[b] = w32b
        w_bcr = A(w32b, 0, [[0, R], [1, K]])

        # X8 [128, 64] col = k*8 + t (t: 0:4 rw, 4:8 w*rw)
        X8 = pw.tile([128, 64], f32r, tag=f"x8_{b}")
        V.tensor_copy(A(X8, 0, [[1, R], [8, K]]),
                      A(rwT, b*32, [[8, R], [1, K]]))
        V.tensor_tensor(A(X8, 4, [[1, R], [8, K]]),
                        A(rwT, b*32, [[8, R], [1, K]]), w_bcr, ALU.mult)
        X8s[b] = X8

        # wdots + DOTSb + WPAb
        PWtb = psc.tile([128, 32], f32, tag="pwtb", bufs=2)
        P.tensor_tensor(PWtb[:], A(rwT, b*32, [[8, R], [1, K]]),
                        w_bcr, ALU.mult)
        V.tensor_reduce(PARTSD[:, b*8+4:b*8+8],
                        A(PWtb, 0, [[8, R], [1, 8]]), AX.X, ALU.add)
        psD = psmall.tile([128, 8], f32, tag="sp")
        T.matmul(psD[:], ones, PARTSD[:, b*8:(b+1)*8], start=True, stop=True)
        DOTSb = pw.tile([128, 8], f32, tag=f"dots_{b}")
        V.tensor_copy(DOTSb[:], psD[:])
        DOTSs[b] = DOTSb
        WPAb = pw.tile([128, 8], f32, tag=f"wpa_{b}")
        P.tensor_tensor(WPAb[:], w32b[:], p32[:, b*8:(b+1)*8], ALU.mult)
        WPAs[b] = WPAb

    def pre_batch_b(b):
        Mb = Mbs[b]
        w32b = W32s[b]
        # ---- Mn = Mb - Mb*(w x ev) + (w x wv), halves on V and P ---------
        Mn = pmem.tile([128, K*D], f32, tag="mn", name=f"mn{b}", bufs=4)
        Mns[b] = Mn
        HD = K * D // 2
        mt1 = psc.tile([128, K*D], f32, tag="mt1", bufs=2)
        mt2 = psc.tile([128, K*D], f32, tag="mt2", bufs=2)
        for h, E in ((0, P), (1, V)):
            sl = slice(h*HD, (h+1)*HD)
            w_bc = A(w32b, 4*h, [[1, K//2], [0, D]])
            ev_bc = M(ROWR, b*128, [[BB*128, 128], [0, K//2], [1, D]])
            wv_bc = M(ROWR, b*128 + 64, [[BB*128, 128], [0, K//2], [1, D]])
            E.tensor_tensor(mt1[:, sl], w_bc, ev_bc, ALU.mult)
            E.tensor_tensor(mt1[:, sl], mt1[:, sl], Mb[:, sl], ALU.mult)
            E.tensor_tensor(Mn[:, sl], Mb[:, sl], mt1[:, sl], ALU.subtract)
            E.tensor_tensor(mt2[:, sl], w_bc, wv_bc, ALU.mult)
            E.tensor_tensor(Mn[:, sl], Mn[:, sl], mt2[:, sl], ALU.add)
        # ---- MnTS + read-content proj + RCb ------------------------------
        MnTS = pmem.tile([128, C], f32, tag="mnts", name=f"mnts{b}", bufs=2)
        for g in range(2):
            psg = ptp.tile([64, 512], f32, tag="tp", bufs=2, name=f"ntp{b}_{g}")
            for q in range(4):
                k = g*4 + q
                T.transpose(psg[:, q*128:(q+1)*128], Mn[:, k*D:(k+1)*D],
                            eye[:])
            (V.tensor_copy if g == 0 else S.copy)(
                MnTS[0:64, g*512:(g+1)*512], psg[:])
        S.square(M(MnTS, 64*C, [[C, 64], [1, C]]), MnTS[0:64, :])
        psR = psmall.tile([128, 64], f32, tag="sp")
        for k in range(K):
            T.matmul(psR[:, k*8:k*8+8], MnTS[:, k*128:(k+1)*128],
                     RHSR[:, b*8:(b+1)*8], start=True, stop=True)
        MNRb = psc.tile([128, 32], f32, tag="mnrb", bufs=2)
        S.activation(MNRb[:], A(psR, 4, [[1, R], [8, K]]), AF.Ln)
        S.activation(MNRb[:], MNRb[:], AF.Exp, scale=0.5)
        V.tensor_tensor(MNRb[:], MNRb[:],
                        A(SALL_t, b*NS+S_KNR, [[1, R], [0, K]]), ALU.mult)
        V.tensor_scalar(MNRb[:], MNRb[:], EPS, None, ALU.add)
        V.reciprocal(MNRb[:], MNRb[:])
        V.tensor_tensor(MNRb[:], A(psR, 0, [[1, R], [8, K]]),
                        MNRb[:], ALU.mult)
        V.tensor_tensor(MNRb[:], MNRb[:],
                        A(SALL_t, b*NS+S_RS, [[1, R], [0, K]]), ALU.mult)
        RCb = pw.tile([128, 32], f32, tag="rcb", bufs=2)
        S.activation(RCb[:], MNRb[:], AF.Exp)
        psZr = psmall.tile([128, 32], f32, tag="sp")
        T.matmul(psZr[:], ones, RCb[:], start=True, stop=True)
        Z4 = psc.tile([128, 4], f32, tag="z4", bufs=2)
        V.tensor_reduce(Z4[:], A(psZr, 0, [[8, 4], [1, 8]]), AX.X, ALU.add)
        V.reciprocal(Z4[:], Z4[:])
        V.tensor_tensor(RCb[:], RCb[:], A(Z4, 0, [[1, R], [0, K]]),
                        ALU.mult)
        RCbs[b] = RCb

    LTss = {}
    PSLs = {}

    def link_tp(b):
        if LN_all[b] is None:
            LN_all[b] = load_ln(b)
        LN = LN_all[b]
        X8r = X8s[b][:]
        psL = plp.tile([128, 128], f32, tag="psL", name=f"psL{b}")
        PSLs[b] = psL
        LTs = []
        for ih in range(K):
            psT = ptp.tile([128, 1024], f32r, tag="psT", bufs=2,
                           name=f"psT{b}_{ih}")
            for jh in range(K):
                T.transpose(psT[:, jh*128:(jh+1)*128],
                            LN[ih][:, jh*128:(jh+1)*128], eyer)
            lt = plt.tile([128, 1024], f32r, tag="lt", bufs=8,
                          name=f"lt{b}_{ih}")
            (V.tensor_copy if ih % 2 == 0 else S.copy)(lt[:], psT[:])
            LTs.append(lt)
        LTss[b] = LTs
        for ih in range(K):
            for jh in range(K):
                T.matmul(psL[:, jh*8:(jh+1)*8],
                         LN[ih][:, jh*128:(jh+1)*128],
                         M(X8r, ih*8, [[64, 128], [1, 8]]),
                         start=(ih == 0), stop=(ih == K-1))
        if b + 2 < BB and LN_all[b + 2] is None:
            LN_all[b + 2] = load_ln(b + 2)

    def link_mm_post(b):
        LTs = LTss[b]
        X8r = X8s[b][:]
        w32b = W32s[b]
        psL = PSLs[b]
        for ih in range(K):
            for jh in range(K):
                T.matmul(psL[:, 64+ih*8:64+(ih+1)*8],
                         LTs[ih][:, jh*128:(jh+1)*128],
                         M(X8r, jh*8, [[64, 128], [1, 8]]),
                         start=(jh == 0), stop=(jh == K-1))
        BL = pw.tile([128, 128], f32, tag="bl", bufs=2)
        FL = BL[:, 64:128]
        V.tensor_copy(BL[:, 0:64], psL[:, 0:64])
        S.copy(BL[:, 64:128], psL[:, 64:128])

        # ---- corrections -> RAWB [128, 64]: cols 0:32 bwd, 32:64 fwd -----
        rv_b = A(rwT, b*32, [[8, R], [1, K]])          # rw (r,k)
        w_bck = A(w32b, 0, [[0, R], [1, K]])
        p_bc = A(p32, b*8, [[0, R], [1, K]])
        wpa_bc = A(WPAs[b], 0, [[0, R], [1, K]])
        pd_bc = A(DOTSs[b], 0, [[1, R], [0, K]])
        wd_bc = A(DOTSs[b], 4, [[1, R], [0, K]])
        BLr = A(BL, 0, [[1, R], [8, K]])               # rwL
        BLw = A(BL, 4, [[1, R], [8, K]])               # wrwL
        FLr = A(FL, 0, [[1, R], [8, K]])               # Lrw
        FLw = A(FL, 4, [[1, R], [8, K]])               # Lwrw
        RAWB = psc.tile([128, 64], f32, tag="rawb", bufs=2)
        DT = psc.tile([128, 32], f32, tag="dt", bufs=2)
        P.tensor_tensor(DT[:], rv_b, wpa_bc, ALU.mult)
        s1 = psc.tile([128, 32], f32, tag="s1", bufs=2)
        s2 = psc.tile([128, 32], f32, tag="s2", bufs=2)
        P.tensor_tensor(s1[:], BLr, w_bck, ALU.mult)
        P.tensor_tensor(s2[:], BLr, BLw, ALU.subtract)
        P.tensor_tensor(s2[:], s2[:], s1[:], ALU.subtract)
        P.tensor_tensor(s1[:], p_bc, wd_bc, ALU.mult)
        P.tensor_tensor(s2[:], s2[:], s1[:], ALU.add)
        P.tensor_tensor(RAWB[:, 0:32], s2[:], DT[:], ALU.subtract)
        s3 = psc.tile([128, 32], f32, tag="s3", bufs=2)
        s4 = psc.tile([128, 32], f32, tag="s4", bufs=2)
        V.tensor_tensor(s3[:], FLr, w_bck, ALU.mult)
        V.tensor_tensor(s3[:], FLr, s3[:], ALU.subtract)
        V.tensor_tensor(s3[:], s3[:], FLw, ALU.subtract)
        V.tensor_tensor(s4[:], w_bck, pd_bc, ALU.mult)
        V.tensor_tensor(s3[:], s3[:], s4[:], ALU.add)
        V.tensor_tensor(RAWB[:, 32:64], s3[:], DT[:], ALU.subtract)

        # ---- sharpen + mix + read vectors --------------------------------
        P.tensor_scalar(RAWB[:], RAWB[:], 0.0, None, ALU.max)
        S.activation(RAWB[:], RAWB[:], AF.Ln, bias=epsc[:])
        P.tensor_tensor(RAWB[:], RAWB[:],
                        A(SALL_t, b*NS+S_MS, [[0, 2], [1, R], [0, K]]),
                        ALU.mult)
        S.activation(RAWB[:], RAWB[:], AF.Exp)
        psZb = psmall.tile([128, 64], f32, tag="sp")
        T.matmul(psZb[:], ones, RAWB[:], start=True, stop=True)
        Z8b = psc.tile([128, 8], f32, tag="z8b", bufs=2)
        V.tensor_reduce(Z8b[:], A(psZb, 0, [[8, 8], [1, 8]]), AX.X, ALU.add)
        V.reciprocal(Z8b[:], Z8b[:])
        P.tensor_tensor(RAWB[:], RAWB[:],
                        A(Z8b, 0, [[4, 2], [1, R], [0, K]]), ALU.mult)
        RWTSb = pw.tile([128, 32], f32, tag="rwtsb", bufs=2)
        mxb = psc.tile([128, 32], f32, tag="mxb", bufs=2)
        P.tensor_tensor(RWTSb[:], RCbs[b][:],
                        A(SALL_t, b*NS+S_PIE+1, [[3, R], [0, K]]), ALU.mult)
        P.tensor_tensor(mxb[:], RAWB[:, 32:64],
                        A(SALL_t, b*NS+S_PIE+2, [[3, R], [0, K]]), ALU.mult)
        P.tensor_tensor(RWTSb[:], RWTSb[:], mxb[:], ALU.add)
        V.tensor_tensor(mxb[:], RAWB[:, 0:32],
                        A(SALL_t, b*NS+S_PIE+0, [[3, R], [0, K]]), ALU.mult)
        V.tensor_add(RWTSb[:], RWTSb[:], mxb[:])
        psRV = psmall.tile([BB, D], f32, tag="sp")
        for k in range(K):
            T.matmul(psRV[:], M(RWTSb[:], k, [[32, 128], [8, R]]),
                     Mns[b][:, k*D:(k+1)*D], start=(k == 0), stop=(k == K-1))
        rvb = pw.tile([BB, D], f32, tag="rvb", bufs=2)
        S.copy(rvb[:], psRV[:])
        SY.dma_start(out_rv[b], rvb[:])

    # pipelined emission: transposes chase each batch's DMA stream;
    # stage-B (w32-gated) work interleaves behind the transposes
    pre_batch_a(0)
    pre_batch_a(1)
    link_tp(0)
    pre_batch_b(0)
    pre_batch_a(2)
    for b in range(BB):
        link_mm_post(b)
        if b + 1 < BB:
            link_tp(b + 1)
            pre_batch_b(b + 1)
        if b + 3 < BB:
            pre_batch_a(b + 3)


# ======================= host-side wrapper =======================
_CACHE = {}

def _get_program():
    if "nc" not in _CACHE:
        _CACHE["nc"] = build_program()
    return _CACHE["nc"]


def _in_maps(x, W_if, b_if, memory, usage, link, precedence,
             prev_read_weights, prev_write_weights):
    f4 = np.float32
    cpack = np.zeros((128, 768), f4)
    cpack[:, 0:128] = np.eye(128, dtype=f4)
    cpack[:, 128:256] = 1.0
    for bb in range(BB):
        cpack[bb, 256+bb*128:256+(bb+1)*128] = 1.0
    in_maps = []
    for c in range(NC):
        s = slice(c*BB, (c+1)*BB)
        in_maps.append({
            "x": np.ascontiguousarray(x[s], f4),
            "W_if": np.ascontiguousarray(W_if, f4),
            "b_if": np.ascontiguousarray(b_if, f4),
            "memory": np.ascontiguousarray(memory[s], f4),
            "usage": np.ascontiguousarray(usage[s], f4),
            "link": np.ascontiguousarray(link[s].reshape(BB, C, C), f4),
            "precedence": np.ascontiguousarray(precedence[s].reshape(BB, C), f4),
            "prw": np.ascontiguousarray(prev_read_weights[s], f4),
            "pww": np.ascontiguousarray(prev_write_weights[s].reshape(BB, C), f4),
            "cpack": cpack,
        })
    return in_maps


def kernel(x, W_if, b_if, memory, usage, link, precedence,
           prev_read_weights, prev_write_weights):
    from concourse.bass_utils import run_bass_kernel_spmd
    nc = _get_program()
    in_maps = _in_maps(x, W_if, b_if, memory, usage, link, precedence,
                       prev_read_weights, prev_write_weights)
    res = run_bass_kernel_spmd(nc, in_maps, list(range(NC)))
    out = np.concatenate([res.results[c]["read_vectors"] for c in range(NC)],
                         axis=0)
    return out
